# revision 1
# baseline (speedup 1.0000x reference)
"""Causal self-attention with RoPE, tensor-parallel over 8 NeuronCores.

Sharding: 8 cores = 2 (batch) x 4 (head-groups of 4 heads).
Each core computes q/k/v projections for its 4 heads, RoPE, causal
softmax(QK^T)V, and a partial output projection (its rows of Wo).
Host sums the 4 partials per batch and adds bo.

Device layouts (chosen so no on-device transposes are needed):
  xT      [D, S]        x[b] transposed (host)
  q,k     [128(hd), h, S]  "T-layout", head dim on partitions
  v       [128(s), S/128, h*128]  natural layout (operand roles swapped)
  scores  [128(k), 512(q)] transposed scores; softmax denominator via
          ones-vector matmul on PE; masking multiplicative with exp(mask).
  out     [S, D] natural (q back on partitions via final matmul orientation)
"""

import math
import os
import sys

sys.path.insert(0, "/opt/trn_rl_repo")

import numpy as np

from contextlib import nullcontext as _nullcm

import concourse.bass as bass
import concourse.tile as tile
from concourse import bacc, mybir
from concourse.bass import ds, ts

B, S, D = 2, 2048, 2048
H, HD = 16, 128
ROPE_BASE = 10000.0
N_CORES = 8
N_GROUPS = 4          # head groups (tensor-parallel axis)
H_LOC = H // N_GROUPS  # heads per core
MM_F32R = os.environ.get("KBENCH_MM_DTYPE", "f32r") == "f32r"

QB = 512   # query block (free dim of score tiles)
KB = 128   # key block (partition dim of score tiles)


def build_core_program(Sn, Dm, h_loc, kb_plan, n_masks, mm_f32r):
    """One core's program (SPMD-shared). kb_plan[qq] = [(kb, mask_idx|None)]."""
    W = h_loc * HD           # local width of Wq/Wk/Wv (columns), Wo (rows)
    KK = Dm // 128           # contraction subtiles for projections
    NSB = Sn // QB           # 512-wide s blocks
    NMB = Sn // KB           # 128-wide s blocks
    f32 = mybir.dt.float32
    mdt = mybir.dt.float32r if mm_f32r else f32

    nc = bacc.Bacc("TRN2", target_bir_lowering=False, debug=False,
                   enable_asserts=True, num_devices=N_CORES)

    xT = nc.dram_tensor("xT", [Dm, Sn], mdt, kind="ExternalInput").ap()
    wq = nc.dram_tensor("wq", [Dm, W], mdt, kind="ExternalInput").ap()
    wk = nc.dram_tensor("wk", [Dm, W], mdt, kind="ExternalInput").ap()
    wv = nc.dram_tensor("wv", [Dm, W], mdt, kind="ExternalInput").ap()
    wo = nc.dram_tensor("wo", [W, Dm], mdt, kind="ExternalInput").ap()
    bq = nc.dram_tensor("bq", [HD, h_loc], f32, kind="ExternalInput").ap()
    bk = nc.dram_tensor("bk", [HD, h_loc], f32, kind="ExternalInput").ap()
    bv = nc.dram_tensor("bv", [1, W], f32, kind="ExternalInput").ap()
    cos2 = nc.dram_tensor("cos2", [HD, Sn], f32, kind="ExternalInput").ap()
    sinS = nc.dram_tensor("sinS", [HD, Sn], f32, kind="ExternalInput").ap()
    if n_masks:
        pmask = nc.dram_tensor("pmask", [n_masks, KB, QB], f32,
                               kind="ExternalInput").ap()
    out = nc.dram_tensor("out", [Sn, Dm], f32, kind="ExternalOutput").ap()

    scale = 1.0 / math.sqrt(HD)

    with tile.TileContext(nc) as tc:
        with (
            tc.tile_pool(name="const", bufs=1) as cpool,
            tc.tile_pool(name="big", bufs=1) as big,
        ):
            # constants (cos/sin/mask DMAs deferred to later program points
            # so the A1-critical loads win queue priority)
            cos2_sb = cpool.tile([HD, Sn], f32, tag="cos2")
            sinS_sb = cpool.tile([HD, Sn], f32, tag="sinS")
            bq_sb = cpool.tile([HD, h_loc], f32, tag="bq")
            nc.sync.dma_start(bq_sb[:], bq[:])
            bk_sb = cpool.tile([HD, h_loc], f32, tag="bk")
            nc.sync.dma_start(bk_sb[:], bk[:])
            bv_sb = cpool.tile([1, W], f32, tag="bv")
            nc.sync.dma_start(bv_sb[:], bv[:])
            ones_f = cpool.tile([HD, HD], f32, tag="ones_f")
            nc.gpsimd.memset(ones_f[:], 1.0)
            ones_c = cpool.tile([HD, 1], mdt, tag="ones_c")   # denominator lhsT
            nc.vector.tensor_copy(ones_c[:], ones_f[:, 0:1])
            bvb = cpool.tile([HD, W], f32, tag="bvb")         # bv bcast on parts
            nc.gpsimd.partition_broadcast(bvb[:], bv_sb[:])

            # persistent activations (q, k stay resident; v, attn spill)
            qb_sb = big.tile([HD, h_loc, Sn], mdt, tag="qb")
            kb_sb = big.tile([HD, h_loc, Sn], mdt, tag="kb")

            # (v allocated after A1 so it reuses Wq/Wk's freed region)
            with _nullcm():
                # ---- Phase A1: q,k projections + RoPE in the DVE shadow ----
                # Wq, Wk resident; x streamed once; per-(tensor,head,sblock)
                # rope chunks run on DVE right after each eviction, while PE
                # continues projecting. A2's pools are opened first so their
                # SBUF region is disjoint from A1's (no stack-reuse dep on
                # A1's trailing rope work).
                a1_cm = [tc.tile_pool(name="wa", bufs=1),
                         tc.tile_pool(name="xa", bufs=4),
                         tc.tile_pool(name="swp", bufs=4)]
                wpool, xpool, spool = (cm.__enter__() for cm in a1_cm)
                if True:
                    psa_cm = tc.tile_pool(name="psa", bufs=1, space="PSUM")
                    psa = psa_cm.__enter__()
                    wq_sb = wpool.tile([128, KK, W], mdt, tag="wqr")
                    wk_sb = wpool.tile([128, KK, W], mdt, tag="wkr")
                    # first contraction tile on the x queue (fast start);
                    # rest + rope tables on the ACT queue.
                    nc.sync.dma_start(wq_sb[:, 0, :], wq[ts(0, 128), :])
                    nc.scalar.dma_start(wk_sb[:, 0, :], wk[ts(0, 128), :])
                    for kk in range(1, KK):
                        nc.scalar.dma_start(wq_sb[:, kk, :], wq[ts(kk, 128), :])
                        nc.scalar.dma_start(wk_sb[:, kk, :], wk[ts(kk, 128), :])
                    nc.scalar.dma_start(cos2_sb[:], cos2[:])
                    nc.scalar.dma_start(sinS_sb[:], sinS[:])

                    def rope_chunk(srct, h, sb):
                        sl = ts(sb, QB)
                        sw = spool.tile([HD, QB], mdt, tag="sw")
                        nc.scalar.dma_start(sw[:64], srct[64:128, h, sl])
                        nc.scalar.dma_start(sw[64:128], srct[:64, h, sl])
                        nc.vector.tensor_mul(srct[:, h, sl], srct[:, h, sl],
                                             cos2_sb[:, sl])
                        nc.vector.tensor_mul(sw[:], sw[:], sinS_sb[:, sl])
                        nc.vector.tensor_add(srct[:, h, sl], srct[:, h, sl],
                                             sw[:])

                    for sb in range(NSB):
                        q_ps = [psa.tile([HD, QB], f32, tag=f"qk{h}",
                                         name=f"q_ps{h}") for h in range(h_loc)]
                        k_ps = [psa.tile([HD, QB], f32, tag=f"kk{h}",
                                         name=f"k_ps{h}") for h in range(h_loc)]
                        for kk in range(KK):
                            xt = xpool.tile([128, QB], mdt, tag="xt")
                            nc.sync.dma_start(xt[:], xT[ts(kk, 128), ts(sb, QB)])
                            for h in range(h_loc):
                                nc.tensor.matmul(q_ps[h][:],
                                                 wq_sb[:, kk, ts(h, HD)],
                                                 xt[:], start=(kk == 0),
                                                 stop=(kk == KK - 1))
                                nc.tensor.matmul(k_ps[h][:],
                                                 wk_sb[:, kk, ts(h, HD)],
                                                 xt[:], start=(kk == 0),
                                                 stop=(kk == KK - 1))
                        for h in range(h_loc):
                            nc.vector.tensor_scalar_add(
                                qb_sb[:, h, ts(sb, QB)], q_ps[h][:],
                                bq_sb[:, h, None])
                            nc.vector.tensor_scalar_add(
                                kb_sb[:, h, ts(sb, QB)], k_ps[h][:],
                                bk_sb[:, h, None])
                        for h in range(h_loc):
                            rope_chunk(qb_sb, h, sb)
                            rope_chunk(kb_sb, h, sb)

                    # --------- Phase A2: v projection (v stays in SBUF) -----
                    # close A1 pools; v + A2 stream pools land in Wq/Wk's
                    # freed region (released at A1's last matmul, NOT at the
                    # rope tail); psv reuses psa's banks (released by the
                    # quick evictions).
                    psa_cm.__exit__(None, None, None)
                    for cm in reversed(a1_cm):
                        cm.__exit__(None, None, None)
                    vpool_cm = tc.tile_pool(name="vpool", bufs=1)
                    vpool = vpool_cm.__enter__()
                    v_sb = vpool.tile([KB, NMB, W], mdt, tag="v")
                    a2_cm = [tc.tile_pool(name="wvs", bufs=4),
                             tc.tile_pool(name="xv", bufs=4)]
                    wpool, xpool = (cm.__enter__() for cm in a2_cm)
                    psv_cm = tc.tile_pool(name="psv", bufs=2, space="PSUM")
                    psv = psv_cm.__enter__()
                    nm = QB // KB
                    for sb in range(NSB):
                        v_ps = [psv.tile([KB, W], f32, tag=f"v{m}",
                                         name=f"v_ps{m}") for m in range(nm)]
                        for kk in range(KK):
                            xt = xpool.tile([128, QB], mdt, tag="xt")
                            nc.sync.dma_start(xt[:], xT[ts(kk, 128), ts(sb, QB)])
                            wv_t = wpool.tile([128, W], mdt, tag="wvt")
                            nc.gpsimd.dma_start(wv_t[:], wv[ts(kk, 128), :])
                            for m in range(nm):
                                nc.tensor.matmul(v_ps[m][:], xt[:, ts(m, KB)],
                                                 wv_t[:],
                                                 start=(kk == 0),
                                                 stop=(kk == KK - 1))
                        for m in range(nm):
                            nc.vector.scalar_tensor_tensor(
                                v_sb[:, sb * nm + m, :], v_ps[m][:], 0.0,
                                bvb[:], op0=mybir.AluOpType.add,
                                op1=mybir.AluOpType.add)
                    psv_cm.__exit__(None, None, None)
                    for cm in reversed(a2_cm):
                        cm.__exit__(None, None, None)

                # ------------- Phase B + C interleaved per q-block ----------
                with (
                    tc.tile_pool(name="mk", bufs=1) as mkpool,
                    tc.tile_pool(name="wc", bufs=1) as wcpool,
                    tc.tile_pool(name="pb", bufs=6) as ppool,
                    tc.tile_pool(name="nb", bufs=2) as npool,
                    tc.tile_pool(name="ac", bufs=2) as acache,
                    tc.tile_pool(name="oc", bufs=3) as opool,
                    tc.tile_pool(name="pss", bufs=3, space="PSUM") as pss,
                    tc.tile_pool(name="pso", bufs=2, space="PSUM") as pso,
                    tc.tile_pool(name="psl", bufs=1, space="PSUM") as psl,
                    tc.tile_pool(name="psc", bufs=2, space="PSUM") as psc,
                ):
                    if n_masks:
                        mask_sb = mkpool.tile([KB, n_masks, QB], f32,
                                              tag="mask")
                        nc.gpsimd.dma_start(
                            mask_sb[:], pmask.rearrange("n p q -> p n q"))
                    wo_sb = wcpool.tile([HD, h_loc, Dm], mdt, tag="wo")
                    for h in range(h_loc):
                        nc.sync.dma_start(wo_sb[:, h, :],
                                          wo[ds(h * HD, HD), :])
                    nm = QB // KB
                    for qq in sorted(range(NSB),
                                     key=lambda q: -len(kb_plan[q])):
                        plan = kb_plan[qq]
                        act = acache.tile([HD, h_loc, QB], mdt, tag="act")
                        for h in range(h_loc):
                            outp = pso.tile([HD, QB], f32, tag="o")
                            lp = psl.tile([1, QB], f32, tag="l")
                            last = len(plan) - 1
                            for i, (kb, mi) in enumerate(plan):
                                sp = pss.tile([KB, QB], f32, tag="s")
                                nc.tensor.matmul(sp[:], kb_sb[:, h, ts(kb, KB)],
                                                 qb_sb[:, h, ts(qq, QB)],
                                                 start=True, stop=True)
                                pt = ppool.tile([KB, QB], mdt, tag="p")
                                nc.scalar.activation(
                                    pt[:], sp[:],
                                    mybir.ActivationFunctionType.Exp,
                                    bias=0.0, scale=scale)
                                if mi is not None:
                                    nc.vector.tensor_mul(pt[:], pt[:],
                                                         mask_sb[:, mi, :])
                                nc.tensor.matmul(outp[:],
                                                 v_sb[:, kb, ts(h, HD)],
                                                 pt[:], start=(i == 0),
                                                 stop=(i == last))
                                nc.tensor.matmul(lp[:], ones_c[:], pt[:],
                                                 start=(i == 0),
                                                 stop=(i == last))
                            rec = npool.tile([1, QB], f32, tag="rec")
                            nc.vector.reciprocal(rec[:], lp[:])
                            recb = npool.tile([HD, QB], f32, tag="recb")
                            nc.gpsimd.partition_broadcast(recb[:], rec[:])
                            nc.vector.scalar_tensor_tensor(
                                act[:, h, :], outp[:], 1.0, recb[:],
                                op0=mybir.AluOpType.mult,
                                op1=mybir.AluOpType.mult)
                        # output projection for this q-block's rows
                        for mi_ in range(nm):
                            m = qq * nm + mi_
                            for n in range(Dm // QB):
                                op = psc.tile([KB, QB], f32, tag="c")
                                for h in range(h_loc):
                                    nc.tensor.matmul(op[:],
                                                     act[:, h, ts(mi_, KB)],
                                                     wo_sb[:, h, ts(n, QB)],
                                                     start=(h == 0),
                                                     stop=(h == h_loc - 1))
                                ot = opool.tile([KB, QB], f32, tag="ot")
                                nc.vector.tensor_copy(ot[:], op[:])
                                nc.sync.dma_start(out[ts(m, KB), ts(n, QB)],
                                                  ot[:])
                vpool_cm.__exit__(None, None, None)

    nc.compile()
    return nc


# ---------------------------------------------------------------------------
# Host side
# ---------------------------------------------------------------------------

def _rope_tables(Sn):
    inv = 1.0 / (ROPE_BASE ** (np.arange(0, HD, 2, dtype=np.float32) / HD))
    ang = np.arange(Sn, dtype=np.float32)[:, None] * inv[None, :]
    cosT = np.cos(ang).T.astype(np.float32)          # [64, S]
    sinT = np.sin(ang).T.astype(np.float32)
    cos2 = np.concatenate([cosT, cosT], 0)           # [128, S]
    sinS = np.concatenate([-sinT, sinT], 0)
    return np.ascontiguousarray(cos2), np.ascontiguousarray(sinS)


def _classify_mask(mask, Sn):
    """-> (kb_plan, mask_tiles). kb_plan[qq] = [(kb, mask_idx|None)]."""
    nq, nk = Sn // QB, Sn // KB
    plan = []
    uniq = {}
    tiles = []
    for qq in range(nq):
        row = []
        for kb in range(nk):
            sub = mask[qq * QB:(qq + 1) * QB, kb * KB:(kb + 1) * KB]
            if sub.max() <= -200.0:
                continue                      # exp() == 0 exactly: skip
            if np.all(sub == 0.0):
                row.append((kb, None))
                continue
            t = np.ascontiguousarray(np.exp(sub.astype(np.float64))
                                     .astype(np.float32).T)  # [KB, QB]
            key = t.tobytes()
            if key not in uniq:
                uniq[key] = len(tiles)
                tiles.append(t)
            row.append((kb, uniq[key]))
        plan.append(row)
    return plan, tiles


_CACHE = {}


def _get_runner(plan_key, Sn, Dm, h_loc, kb_plan, n_masks):
    if plan_key in _CACHE:
        return _CACHE[plan_key]
    nc = build_core_program(Sn, Dm, h_loc, kb_plan, n_masks, MM_F32R)
    runner = _make_pjrt_runner(nc, N_CORES)
    _CACHE[plan_key] = runner
    return runner


def _make_pjrt_runner(nc, n_cores):
    """Persistent jitted SPMD executor (replicates bass2jax.run_bass_via_pjrt
    multi-core path, but reusable across calls for stable timing)."""
    import jax
    from jax.sharding import Mesh, PartitionSpec
    from jax.experimental.shard_map import shard_map
    from concourse.bass2jax import (_bass_exec_p, install_neuronx_cc_hook,
                                    partition_id_tensor)

    install_neuronx_cc_hook()
    pname = nc.partition_id_tensor.name if nc.partition_id_tensor else None
    in_names, out_names, out_avals, zero_outs = [], [], [], []
    for alloc in nc.m.functions[0].allocations:
        if not isinstance(alloc, mybir.MemoryLocationSet):
            continue
        name = alloc.memorylocations[0].name
        if alloc.kind == "ExternalInput":
            if name != pname:
                in_names.append(name)
        elif alloc.kind == "ExternalOutput":
            shape = tuple(alloc.tensor_shape)
            dtype = mybir.dt.np(alloc.dtype)
            out_names.append(name)
            out_avals.append(jax.core.ShapedArray(shape, dtype))
            zero_outs.append(np.zeros(shape, dtype))
    n_params = len(in_names)
    all_names = in_names + out_names
    if pname is not None:
        all_names = all_names + [pname]

    def _body(*args):
        operands = list(args)
        if pname is not None:
            operands.append(partition_id_tensor())
        outs = _bass_exec_p.bind(
            *operands, out_avals=tuple(out_avals), in_names=tuple(all_names),
            out_names=tuple(out_names), lowering_input_output_aliases=(),
            sim_require_finite=True, sim_require_nnan=True, nc=nc)
        return tuple(outs)

    devices = jax.devices()[:n_cores]
    mesh = Mesh(np.asarray(devices), ("core",))
    nin = n_params + len(out_names)
    jfn = jax.jit(shard_map(_body, mesh=mesh,
                            in_specs=(PartitionSpec("core"),) * nin,
                            out_specs=(PartitionSpec("core"),) * len(out_names),
                            check_rep=False),
                  keep_unused=True)

    def run(in_maps):
        concat = [np.concatenate([np.asarray(m[nm]) for m in in_maps], axis=0)
                  for nm in in_names]
        zeros = [np.zeros((n_cores * z.shape[0], *z.shape[1:]), z.dtype)
                 for z in zero_outs]
        outs = jfn(*concat, *zeros)
        return [{nm: np.asarray(outs[i]).reshape(n_cores, *out_avals[i].shape)[c]
                 for i, nm in enumerate(out_names)} for c in range(n_cores)]

    def make_chain(n):
        def _chain(*args):
            ins = list(args[:n_params])
            outs = tuple(args[n_params:])
            for _ in range(n):
                outs = _body(*ins, *outs)
            return outs
        return jax.jit(shard_map(_chain, mesh=mesh,
                                 in_specs=(PartitionSpec("core"),) * nin,
                                 out_specs=(PartitionSpec("core"),)
                                 * len(out_names),
                                 check_rep=False),
                       keep_unused=True)

    run.jfn = jfn
    run.make_chain = make_chain
    run.in_names = in_names
    run.out_names = out_names
    run.zero_outs = zero_outs
    run.nc = nc
    return run


def _prep_in_maps(x, attn_mask, Wq, bq, Wk, bk, Wv, bv, Wo, mask_tiles):
    cos2, sinS = _rope_tables(S)
    Wg = H_LOC * HD
    pm = (np.ascontiguousarray(np.stack(mask_tiles, 0))
          if mask_tiles else None)
    in_maps = []
    for c in range(N_CORES):
        b, g = divmod(c, N_GROUPS)
        cs = slice(g * Wg, (g + 1) * Wg)
        m = {
            "xT": np.ascontiguousarray(x[b].T),
            "wq": np.ascontiguousarray(Wq[:, cs]),
            "wk": np.ascontiguousarray(Wk[:, cs]),
            "wv": np.ascontiguousarray(Wv[:, cs]),
            "wo": np.ascontiguousarray(Wo[cs, :]),
            "bq": np.ascontiguousarray(bq[cs].reshape(H_LOC, HD).T),
            "bk": np.ascontiguousarray(bk[cs].reshape(H_LOC, HD).T),
            "bv": np.ascontiguousarray(bv[cs][None, :]),
            "cos2": cos2,
            "sinS": sinS,
        }
        if pm is not None:
            m["pmask"] = pm
        in_maps.append(m)
    return in_maps


def kernel(x, attn_mask, Wq, bq, Wk, bk, Wv, bv, Wo, bo):
    x = np.asarray(x, dtype=np.float32)
    mask = np.asarray(attn_mask, dtype=np.float32).reshape(S, S)
    kb_plan, mask_tiles = _classify_mask(mask, S)
    plan_key = (tuple(tuple(r) for r in kb_plan), len(mask_tiles), MM_F32R)
    runner = _get_runner(plan_key, S, D, H_LOC, kb_plan, len(mask_tiles))
    in_maps = _prep_in_maps(x, mask, np.asarray(Wq), np.asarray(bq),
                            np.asarray(Wk), np.asarray(bk), np.asarray(Wv),
                            np.asarray(bv), np.asarray(Wo), mask_tiles)
    results = runner(in_maps)
    out = np.empty((B, S, D), np.float32)
    for b in range(B):
        acc = results[b * N_GROUPS]["out"].astype(np.float32).copy()
        for g in range(1, N_GROUPS):
            acc += results[b * N_GROUPS + g]["out"]
        out[b] = acc + np.asarray(bo, np.float32)[None, :]
    return out



# revision 13
# speedup vs baseline: 1.2529x; 1.2529x over previous
"""Causal self-attention with RoPE, tensor-parallel over 8 NeuronCores.

Sharding: 8 cores = 2 (batch) x 4 (head-groups of 4 heads).
Each core computes q/k/v projections for its 4 heads, RoPE, causal
softmax(QK^T)V, and a partial output projection (its rows of Wo).
Host sums the 4 partials per batch and adds bo' = bo + bv @ Wo
(the v-bias contribution commutes through softmax normalization:
softmax(P)(xWv + bv) = softmax(P)(xWv) + bv).

All matmul operands are bf16 (same 1 cycle/column PE rate as f32r but
half the DMA bytes and 2x DVE elementwise); PSUM accumulation is f32.

Device layouts (no on-device transposes):
  xT      [D, S]         x[b] transposed (host)
  q,k     [128(hd), h, S]  head dim on partitions
  v       [128(s), S/128, h*128]  natural layout
  scores  [128(k), 2, 512(q)]  transposed scores in 2-bank PSUM pairs;
          exp on ACT over the 1024-wide pair; causal mask multiplicative
          (exp(mask), width-limited); softmax denominator accumulated on
          DVE in f32 then summed across partitions with one
          gpsimd.partition_all_reduce (no PE matmul, no PSUM bank).
  out     [S, D] bf16
"""

import math
import sys

sys.path.insert(0, "/opt/trn_rl_repo")

import numpy as np

import concourse.bass as bass
import concourse.tile as tile
from concourse import bacc, bass_isa, mybir
from concourse.bass import ds, ts

B, S, D = 2, 2048, 2048
H, HD = 16, 128
ROPE_BASE = 10000.0
N_CORES = 8
N_GROUPS = 4          # head groups (tensor-parallel axis)
H_LOC = H // N_GROUPS  # heads per core

QB = 512   # query block (free dim of score tiles)
KB = 128   # key block (partition dim of score tiles)


def build_core_program(Sn, Dm, h_loc, kb_plan, mask_w):
    """One core's program (SPMD-shared).

    kb_plan[qq] = [(kb, mask_idx|None)]; mask_w[mi] = effective width
    (columns >= mask_w are all-ones in the exp-mask tile, multiply skipped).
    """
    W = h_loc * HD           # local width of Wq/Wk/Wv (columns), Wo (rows)
    KK = Dm // 128           # contraction subtiles for projections
    NSB = Sn // QB           # 512-wide s blocks
    NMB = Sn // KB           # 128-wide s blocks
    n_masks = len(mask_w)
    f32 = mybir.dt.float32
    bf16 = mybir.dt.bfloat16
    scale = 1.0 / math.sqrt(HD)
    AF = mybir.ActivationFunctionType
    ALU = mybir.AluOpType

    nc = bacc.Bacc("TRN2", target_bir_lowering=False, debug=False,
                   enable_asserts=True, num_devices=N_CORES)

    xT = nc.dram_tensor("xT", [Dm, Sn], bf16, kind="ExternalInput").ap()
    wq = nc.dram_tensor("wq", [Dm, W], bf16, kind="ExternalInput").ap()
    wk = nc.dram_tensor("wk", [Dm, W], bf16, kind="ExternalInput").ap()
    wv = nc.dram_tensor("wv", [Dm, W], bf16, kind="ExternalInput").ap()
    wo = nc.dram_tensor("wo", [W, Dm], bf16, kind="ExternalInput").ap()
    bq = nc.dram_tensor("bq", [HD, h_loc], f32, kind="ExternalInput").ap()
    bk = nc.dram_tensor("bk", [HD, h_loc], f32, kind="ExternalInput").ap()
    cos2 = nc.dram_tensor("cos2", [HD, Sn], bf16, kind="ExternalInput").ap()
    sinS = nc.dram_tensor("sinS", [HD, Sn], bf16, kind="ExternalInput").ap()
    if n_masks:
        pmask = nc.dram_tensor("pmask", [n_masks, KB, QB], bf16,
                               kind="ExternalInput").ap()
    out = nc.dram_tensor("out", [Sn, Dm], bf16, kind="ExternalOutput").ap()

    with tile.TileContext(nc) as tc:
        with (
            tc.tile_pool(name="const", bufs=1) as cpool,
            tc.tile_pool(name="big", bufs=1) as big,
            tc.tile_pool(name="wqk", bufs=1) as wqkpool,
        ):
            # persistent tiles
            cos2_sb = cpool.tile([HD, Sn], bf16, tag="cos2")
            sinS_sb = cpool.tile([HD, Sn], bf16, tag="sinS")
            bq_sb = cpool.tile([HD, h_loc], f32, tag="bq")
            bk_sb = cpool.tile([HD, h_loc], f32, tag="bk")
            if n_masks:
                mask_sb = cpool.tile([KB, n_masks, QB], bf16, tag="mask")
            qb_sb = big.tile([HD, h_loc, Sn], bf16, tag="qb")
            kb_sb = big.tile([HD, h_loc, Sn], bf16, tag="kb")
            v_sb = big.tile([KB, NMB, W], bf16, tag="v")
            wo_sb = big.tile([HD, h_loc, Dm], bf16, tag="wo")
            wq_sb = wqkpool.tile([128, KK, W], bf16, tag="wqr")
            wk_sb = wqkpool.tile([128, KK, W], bf16, tag="wkr")

            xpool_cm = tc.tile_pool(name="xp", bufs=3)
            xpool = xpool_cm.__enter__()
            xTr = xT.rearrange("(kk p) s -> p kk s", p=128)
            xq_tiles = []

            def load_x(sb, chunked=False):
                t = xpool.tile([128, KK, QB], bf16, tag="xt",
                               name=f"x{len(xq_tiles)}")
                if chunked:   # 4 DMAs so the first kk-chunk lands fast
                    for c in range(4):
                        nc.sync.dma_start(t[:, ts(c, 4), :],
                                          xTr[:, ts(c, 4), ts(sb, QB)])
                else:
                    nc.sync.dma_start(t[:], xTr[:, :, ts(sb, QB)])
                xq_tiles.append(t)
                return t

            # ---------------- Phase V: v projection ----------------
            with (
                tc.tile_pool(name="wvp", bufs=1) as wvp,
                tc.tile_pool(name="psv", bufs=2, space="PSUM") as psv,
            ):
                wv_sb = wvp.tile([128, KK, W], bf16, tag="wv")
                wvr = wv.rearrange("(kk p) w -> p kk w", p=128)
                # first x block + first chunk of wv race in on two queues
                xts = [load_x(0, chunked=True)]
                for c in range(4):
                    nc.gpsimd.dma_start(wv_sb[:, ts(c, 4), :],
                                        wvr[:, ts(c, 4), :])
                xts.append(load_x(1))
                # preloads for later phases on otherwise-idle queues
                nc.scalar.dma_start(bq_sb[:], bq[:])
                nc.scalar.dma_start(bk_sb[:], bk[:])
                nc.scalar.dma_start(
                    wq_sb[:], wq.rearrange("(kk p) w -> p kk w", p=128))
                nc.scalar.dma_start(
                    wk_sb[:], wk.rearrange("(kk p) w -> p kk w", p=128))
                nc.gpsimd.dma_start(cos2_sb[:], cos2[:])
                nc.gpsimd.dma_start(sinS_sb[:], sinS[:])
                if n_masks:
                    nc.gpsimd.dma_start(
                        mask_sb[:], pmask.rearrange("n p q -> p n q"))
                nc.gpsimd.dma_start(
                    wo_sb[:], wo.rearrange("(h d) c -> d h c", d=HD))

                for sb in range(NSB):
                    if sb >= 2:
                        xts.append(load_x(sb))
                    xt = xts[sb]
                    v_ps = psv.tile([KB, QB // KB, W], f32, tag="vps")
                    for kk in range(KK):
                        for m in range(QB // KB):
                            nc.tensor.matmul(v_ps[:, m, :],
                                             xt[:, kk, ts(m, KB)],
                                             wv_sb[:, kk, :],
                                             start=(kk == 0),
                                             stop=(kk == KK - 1))
                    if sb == NSB - 1:
                        xts.append(load_x(0))   # prefetch QK's first block
                    for m in range(QB // KB):
                        nc.scalar.activation(
                            v_sb[:, sb * (QB // KB) + m, :], v_ps[:, m, :],
                            AF.Copy)

            # ---------------- Phase QK: q/k projection + RoPE ------------
            with (
                tc.tile_pool(name="psa", bufs=2, space="PSUM") as psa,
                tc.tile_pool(name="swp", bufs=4) as swp,
            ):
                def rope(srct, h, sb):
                    sl = ts(sb, QB)
                    sw = swp.tile([HD, QB], bf16, tag="sw")
                    nc.scalar.dma_start(sw[:64], srct[64:128, h, sl])
                    nc.scalar.dma_start(sw[64:128], srct[:64, h, sl])
                    nc.vector.tensor_mul(srct[:, h, sl], srct[:, h, sl],
                                         cos2_sb[:, sl])
                    nc.vector.tensor_mul(sw[:], sw[:], sinS_sb[:, sl])
                    nc.vector.tensor_add(srct[:, h, sl], srct[:, h, sl],
                                         sw[:])

                xts = [xq_tiles[-1]]          # prefetched during phase V
                for sb in range(NSB):
                    if sb >= 1:
                        xts.append(load_x(sb))
                    xt = xts[sb]
                    sl = ts(sb, QB)
                    # two passes (q then k): 4 PSUM banks each, so the bias
                    # evictions of one pass overlap the other pass's matmuls
                    for wsrc, wsb, dst, bias in ((wq, wq_sb, qb_sb, bq_sb),
                                                 (wk, wk_sb, kb_sb, bk_sb)):
                        ps = psa.tile([HD, h_loc, QB], f32, tag="qkps")
                        for kk in range(KK):
                            for h in range(h_loc):
                                nc.tensor.matmul(ps[:, h, :],
                                                 wsb[:, kk, ts(h, HD)],
                                                 xt[:, kk, :],
                                                 start=(kk == 0),
                                                 stop=(kk == KK - 1))
                        for h in range(h_loc):
                            nc.scalar.activation(dst[:, h, sl], ps[:, h, :],
                                                 AF.Identity,
                                                 bias=bias[:, h, None])
                        for h in range(h_loc):
                            rope(dst, h, sb)
            xpool_cm.__exit__(None, None, None)

            # ---------------- Phase B + C: attention + out-proj ----------
            with (
                tc.tile_pool(name="pb", bufs=6) as ppool,
                tc.tile_pool(name="accp", bufs=2) as accp,
                tc.tile_pool(name="lsp", bufs=4) as lsump,
                tc.tile_pool(name="ac", bufs=2) as acache,
                tc.tile_pool(name="oc", bufs=6) as opool,
                tc.tile_pool(name="pss", bufs=2, space="PSUM") as pss,
                tc.tile_pool(name="pso", bufs=2, space="PSUM") as pso,
                tc.tile_pool(name="psc", bufs=2, space="PSUM") as psc,
            ):
                def emit_oproj(qq, act, jobs, drain=False):
                    """One [128q x 512d] out tile: 4 matmuls + copy + store."""
                    if not jobs:
                        return
                    mi_, n = jobs.pop(0)
                    m = qq * (QB // KB) + mi_
                    if drain and (len(jobs) % 2):
                        # final drain: alternate psc with the idle pss banks
                        opw = pss.tile([KB, 2, QB], f32, tag="s", name="opw")
                        op = opw[:, 0, :]
                    else:
                        op = psc.tile([KB, QB], f32, tag="c", name="opc")
                    for h in range(h_loc):
                        nc.tensor.matmul(op[:], act[:, h, ts(mi_, KB)],
                                         wo_sb[:, h, ts(n, QB)],
                                         start=(h == 0),
                                         stop=(h == h_loc - 1))
                    ot = opool.tile([KB, QB], bf16, tag="ot")
                    nc.scalar.activation(ot[:], op[:], AF.Copy)
                    nc.sync.dma_start(out[ts(m, KB), ts(n, QB)], ot[:])

                pending = None     # (qq, act, jobs) from previous q-block
                qorder = sorted(range(NSB), key=lambda q: -len(kb_plan[q]))
                if len(qorder) >= 2:
                    # second-longest first: its q/k blocks finished (and were
                    # roped) earlier, so attention starts without a stall
                    qorder[0], qorder[1] = qorder[1], qorder[0]
                for qq in qorder:
                    plan = kb_plan[qq]
                    groups = [plan[i:i + 2] for i in range(0, len(plan), 2)]
                    act = acache.tile([HD, h_loc, QB], bf16, tag="act")
                    qsl = ts(qq, QB)
                    n_slots = max(1, len(groups) * ((h_loc + 1) // 2))
                    per_slot = ((QB // KB) * (Dm // QB) + n_slots - 1) \
                        // n_slots
                    for hpair in ((0, 1), (2, 3))[:max(1, h_loc // 2)]:
                        hpair = [h for h in hpair if h < h_loc]
                        st = {h: (pso.tile([HD, QB], f32, tag="o",
                                           name=f"o{h}"),
                                  accp.tile([KB, QB], f32, tag="acc",
                                            name=f"acc{h}"))
                              for h in hpair}
                        for gi, grp in enumerate(groups):
                            ng = len(grp)
                            pts = {}
                            for h in hpair:
                                sp2 = pss.tile([KB, 2, QB], f32, tag="s")
                                for s_, (kbi, mi) in enumerate(grp):
                                    nc.tensor.matmul(
                                        sp2[:, s_, :],
                                        kb_sb[:, h, ts(kbi, KB)],
                                        qb_sb[:, h, qsl],
                                        start=True, stop=True)
                                pt = ppool.tile([KB, 2, QB], bf16, tag="p")
                                pts[h] = pt
                                nc.scalar.activation(
                                    pt[:, :ng, :], sp2[:, :ng, :], AF.Exp,
                                    bias=0.0, scale=scale)
                                acc = st[h][1]
                                for s_, (kbi, mi) in enumerate(grp):
                                    if mi is not None:
                                        w = mask_w[mi]
                                        nc.vector.tensor_mul(
                                            pt[:, s_, :w], pt[:, s_, :w],
                                            mask_sb[:, mi, :w])
                                for s_ in range(ng):
                                    if gi == 0 and s_ == 0:
                                        nc.vector.tensor_copy(acc[:],
                                                              pt[:, 0, :])
                                    else:
                                        nc.vector.tensor_add(acc[:], acc[:],
                                                             pt[:, s_, :])
                            # PE filler: out-proj tiles of the previous
                            # q-block run while this group's exps cook
                            if pending is not None:
                                for _ in range(per_slot):
                                    emit_oproj(pending[0], pending[1],
                                               pending[2])
                            last = (gi == len(groups) - 1)
                            for h in hpair:
                                outp = st[h][0]
                                for s_ in range(ng):
                                    nc.tensor.matmul(
                                        outp[:],
                                        v_sb[:, grp[s_][0], ts(h, HD)],
                                        pts[h][:, s_, :],
                                        start=(gi == 0 and s_ == 0),
                                        stop=(last and s_ == ng - 1))
                        for h in hpair:
                            outp, acc = st[h]
                            lsum = lsump.tile([KB, QB], f32, tag="ls")
                            nc.gpsimd.partition_all_reduce(
                                lsum[:], acc[:], 128, bass_isa.ReduceOp.add)
                            recb = lsump.tile([KB, QB], f32, tag="rc")
                            nc.vector.reciprocal(recb[:], lsum[:])
                            nc.vector.scalar_tensor_tensor(
                                act[:, h, :], outp[:], 1.0, recb[:],
                                op0=ALU.mult, op1=ALU.mult)
                    if pending is not None:
                        while pending[2]:
                            emit_oproj(pending[0], pending[1], pending[2])
                    pending = (qq, act,
                               [(mi_, n) for mi_ in range(QB // KB)
                                for n in range(Dm // QB)])
                while pending[2]:
                    emit_oproj(pending[0], pending[1], pending[2],
                               drain=True)

    nc.compile()
    return nc


# ---------------------------------------------------------------------------
# Host side
# ---------------------------------------------------------------------------

def _bf16(a):
    import ml_dtypes
    return np.ascontiguousarray(np.asarray(a).astype(ml_dtypes.bfloat16))


def _rope_tables(Sn):
    inv = 1.0 / (ROPE_BASE ** (np.arange(0, HD, 2, dtype=np.float32) / HD))
    ang = np.arange(Sn, dtype=np.float32)[:, None] * inv[None, :]
    cosT = np.cos(ang).T.astype(np.float32)          # [64, S]
    sinT = np.sin(ang).T.astype(np.float32)
    cos2 = np.concatenate([cosT, cosT], 0)           # [128, S]
    sinS = np.concatenate([-sinT, sinT], 0)
    return _bf16(cos2), _bf16(sinS)


def _classify_mask(mask, Sn):
    """-> (kb_plan, mask_tiles). kb_plan[qq] = [(kb, mask_idx|None)]."""
    nq, nk = Sn // QB, Sn // KB
    plan = []
    uniq = {}
    tiles = []
    for qq in range(nq):
        row = []
        for kb in range(nk):
            sub = mask[qq * QB:(qq + 1) * QB, kb * KB:(kb + 1) * KB]
            if sub.max() <= -200.0:
                continue                      # exp() == 0 exactly: skip
            if np.all(sub == 0.0):
                row.append((kb, None))
                continue
            t = np.ascontiguousarray(np.exp(sub.astype(np.float64))
                                     .astype(np.float32).T)  # [KB, QB]
            key = t.tobytes()
            if key not in uniq:
                uniq[key] = len(tiles)
                tiles.append(t)
            row.append((kb, uniq[key]))
        plan.append(row)
    return plan, tiles


def _mask_widths(tiles):
    """Effective width per tile: columns >= w are exactly 1.0 (skip mult)."""
    ws = []
    for t in tiles:
        not_one = np.where(np.any(t != 1.0, axis=0))[0]
        if len(not_one) == 0:
            ws.append(0)
        else:
            w = int(not_one.max()) + 1
            ws.append(min(QB, ((w + 127) // 128) * 128))
    return ws


_CACHE = {}


def _get_runner(plan_key, Sn, Dm, h_loc, kb_plan, mask_w):
    if plan_key in _CACHE:
        return _CACHE[plan_key]
    nc = build_core_program(Sn, Dm, h_loc, kb_plan, mask_w)
    runner = _make_pjrt_runner(nc, N_CORES)
    _CACHE[plan_key] = runner
    return runner


def _make_pjrt_runner(nc, n_cores):
    """Persistent jitted SPMD executor (replicates bass2jax.run_bass_via_pjrt
    multi-core path, but reusable across calls for stable timing)."""
    import jax
    from jax.sharding import Mesh, PartitionSpec
    from jax.experimental.shard_map import shard_map
    from concourse.bass2jax import (_bass_exec_p, install_neuronx_cc_hook,
                                    partition_id_tensor)

    install_neuronx_cc_hook()
    pname = nc.partition_id_tensor.name if nc.partition_id_tensor else None
    in_names, out_names, out_avals, zero_outs = [], [], [], []
    for alloc in nc.m.functions[0].allocations:
        if not isinstance(alloc, mybir.MemoryLocationSet):
            continue
        name = alloc.memorylocations[0].name
        if alloc.kind == "ExternalInput":
            if name != pname:
                in_names.append(name)
        elif alloc.kind == "ExternalOutput":
            shape = tuple(alloc.tensor_shape)
            dtype = mybir.dt.np(alloc.dtype)
            out_names.append(name)
            out_avals.append(jax.core.ShapedArray(shape, dtype))
            zero_outs.append(np.zeros(shape, dtype))
    n_params = len(in_names)
    all_names = in_names + out_names
    if pname is not None:
        all_names = all_names + [pname]

    def _body(*args):
        operands = list(args)
        if pname is not None:
            operands.append(partition_id_tensor())
        outs = _bass_exec_p.bind(
            *operands, out_avals=tuple(out_avals), in_names=tuple(all_names),
            out_names=tuple(out_names), lowering_input_output_aliases=(),
            sim_require_finite=True, sim_require_nnan=True, nc=nc)
        return tuple(outs)

    devices = jax.devices()[:n_cores]
    mesh = Mesh(np.asarray(devices), ("core",))
    nin = n_params + len(out_names)
    jfn = jax.jit(shard_map(_body, mesh=mesh,
                            in_specs=(PartitionSpec("core"),) * nin,
                            out_specs=(PartitionSpec("core"),) * len(out_names),
                            check_rep=False),
                  keep_unused=True)

    def run(in_maps):
        concat = [np.concatenate([np.asarray(m[nm]) for m in in_maps], axis=0)
                  for nm in in_names]
        zeros = [np.zeros((n_cores * z.shape[0], *z.shape[1:]), z.dtype)
                 for z in zero_outs]
        outs = jfn(*concat, *zeros)
        return [{nm: np.asarray(outs[i]).reshape(n_cores, *out_avals[i].shape)[c]
                 for i, nm in enumerate(out_names)} for c in range(n_cores)]

    def make_chain(n):
        def _chain(*args):
            ins = list(args[:n_params])
            outs = tuple(args[n_params:])
            for _ in range(n):
                outs = _body(*ins, *outs)
            return outs
        return jax.jit(shard_map(_chain, mesh=mesh,
                                 in_specs=(PartitionSpec("core"),) * nin,
                                 out_specs=(PartitionSpec("core"),)
                                 * len(out_names),
                                 check_rep=False),
                       keep_unused=True)

    run.jfn = jfn
    run.make_chain = make_chain
    run.in_names = in_names
    run.out_names = out_names
    run.zero_outs = zero_outs
    run.nc = nc
    return run


def _prep_in_maps(x, attn_mask, Wq, bq, Wk, bk, Wv, bv, Wo, mask_tiles):
    cos2, sinS = _rope_tables(S)
    Wg = H_LOC * HD
    pm = _bf16(np.stack(mask_tiles, 0)) if mask_tiles else None
    in_maps = []
    for c in range(N_CORES):
        b, g = divmod(c, N_GROUPS)
        cs = slice(g * Wg, (g + 1) * Wg)
        m = {
            "xT": _bf16(x[b].T),
            "wq": _bf16(Wq[:, cs]),
            "wk": _bf16(Wk[:, cs]),
            "wv": _bf16(Wv[:, cs]),
            "wo": _bf16(Wo[cs, :]),
            "bq": np.ascontiguousarray(
                np.asarray(bq)[cs].reshape(H_LOC, HD).T.astype(np.float32)),
            "bk": np.ascontiguousarray(
                np.asarray(bk)[cs].reshape(H_LOC, HD).T.astype(np.float32)),
            "cos2": cos2,
            "sinS": sinS,
        }
        if pm is not None:
            m["pmask"] = pm
        in_maps.append(m)
    return in_maps


def kernel(x, attn_mask, Wq, bq, Wk, bk, Wv, bv, Wo, bo):
    x = np.asarray(x, dtype=np.float32)
    mask = np.asarray(attn_mask, dtype=np.float32).reshape(S, S)
    kb_plan, mask_tiles = _classify_mask(mask, S)
    mask_w = _mask_widths(mask_tiles)
    plan_key = (tuple(tuple(r) for r in kb_plan), len(mask_tiles))
    runner = _get_runner(plan_key, S, D, H_LOC, kb_plan, mask_w)
    in_maps = _prep_in_maps(x, mask, np.asarray(Wq), np.asarray(bq),
                            np.asarray(Wk), np.asarray(bk), np.asarray(Wv),
                            np.asarray(bv), np.asarray(Wo), mask_tiles)
    results = runner(in_maps)
    # bo' = bo + bv @ Wo (v-bias commutes through softmax normalization)
    bo_eff = (np.asarray(bo, np.float64)
              + np.asarray(bv, np.float64) @ np.asarray(Wo, np.float64)
              ).astype(np.float32)
    out = np.empty((B, S, D), np.float32)
    for b in range(B):
        acc = results[b * N_GROUPS]["out"].astype(np.float32)
        for g in range(1, N_GROUPS):
            acc = acc + results[b * N_GROUPS + g]["out"].astype(np.float32)
        out[b] = acc + bo_eff[None, :]
    return out


# revision 33
# speedup vs baseline: 1.3406x; 1.0700x over previous
"""Causal self-attention with RoPE, tensor-parallel over 8 NeuronCores.

Sharding: 8 cores = 2 (batch) x 4 (head-groups of 4 heads).
Each core computes q/k/v projections for its 4 heads, RoPE, causal
softmax(QK^T)V, and a partial output projection (its rows of Wo).
Host sums the 4 partials per batch and adds bo' = bo + bv @ Wo
(the v-bias contribution commutes through softmax normalization:
softmax(P)(xWv + bv) = softmax(P)(xWv) + bv).

All matmul operands are bf16 (same 1 cycle/column PE rate as f32r but
half the DMA bytes and 2x DVE elementwise); PSUM accumulation is f32.

Device layouts (no on-device transposes):
  xT      [D, S]         x[b] transposed (host)
  q,k     [128(hd), h, S]  head dim on partitions
  v       [128(s), S/128, h*128]  natural layout
  scores  [128(k), 2, 512(q)]  transposed scores in 2-bank PSUM pairs;
          exp on ACT over the 1024-wide pair; causal mask multiplicative
          (exp(mask), width-limited); softmax denominator accumulated on
          DVE in f32 then summed across partitions with one
          gpsimd.partition_all_reduce (no PE matmul, no PSUM bank).
  out     [S, D] bf16
"""

import math
import sys

sys.path.insert(0, "/opt/trn_rl_repo")

import numpy as np

import concourse.bass as bass
import concourse.tile as tile
from concourse import bacc, bass_isa, mybir
from concourse.bass import ds, ts

B, S, D = 2, 2048, 2048
H, HD = 16, 128
ROPE_BASE = 10000.0
N_CORES = 8
N_GROUPS = 4          # head groups (tensor-parallel axis)
H_LOC = H // N_GROUPS  # heads per core

QB = 512   # query block (free dim of score tiles)
KB = 128   # key block (partition dim of score tiles)


def build_core_program(Sn, Dm, h_loc, kb_plan, mask_w):
    """One core's program (SPMD-shared).

    kb_plan[qq] = [(kb, mask_idx|None)]; mask_w[mi] = effective width
    (columns >= mask_w are all-ones in the exp-mask tile, multiply skipped).
    """
    W = h_loc * HD           # local width of Wq/Wk/Wv (columns), Wo (rows)
    KK = Dm // 128           # contraction subtiles for projections
    NSB = Sn // QB           # 512-wide s blocks
    NMB = Sn // KB           # 128-wide s blocks
    n_masks = len(mask_w)
    f32 = mybir.dt.float32
    bf16 = mybir.dt.bfloat16
    scale = 1.0 / math.sqrt(HD)
    AF = mybir.ActivationFunctionType
    ALU = mybir.AluOpType

    nc = bacc.Bacc("TRN2", target_bir_lowering=False, debug=False,
                   enable_asserts=True, num_devices=N_CORES)

    xT = nc.dram_tensor("xT", [Dm, Sn], bf16, kind="ExternalInput").ap()
    wq = nc.dram_tensor("wq", [Dm, W], bf16, kind="ExternalInput").ap()
    wk = nc.dram_tensor("wk", [Dm, W], bf16, kind="ExternalInput").ap()
    wv = nc.dram_tensor("wv", [Dm, W], bf16, kind="ExternalInput").ap()
    wo = nc.dram_tensor("wo", [W, Dm], bf16, kind="ExternalInput").ap()
    bq = nc.dram_tensor("bq", [HD, h_loc], f32, kind="ExternalInput").ap()
    bk = nc.dram_tensor("bk", [HD, h_loc], f32, kind="ExternalInput").ap()
    cos2 = nc.dram_tensor("cos2", [HD, Sn], bf16, kind="ExternalInput").ap()
    sinS = nc.dram_tensor("sinS", [HD, Sn], bf16, kind="ExternalInput").ap()
    if n_masks:
        pmask = nc.dram_tensor("pmask", [n_masks, KB, QB], bf16,
                               kind="ExternalInput").ap()
    out = nc.dram_tensor("out", [Sn, Dm], bf16, kind="ExternalOutput").ap()

    with tile.TileContext(nc) as tc:
        with (
            tc.tile_pool(name="const", bufs=1) as cpool,
            tc.tile_pool(name="big", bufs=1) as big,
            tc.tile_pool(name="wqk", bufs=1) as wqkpool,
        ):
            # persistent tiles
            cos2_sb = cpool.tile([HD, Sn], bf16, tag="cos2")
            sinS_sb = cpool.tile([HD, Sn], bf16, tag="sinS")
            bq_sb = cpool.tile([HD, h_loc], f32, tag="bq")
            bk_sb = cpool.tile([HD, h_loc], f32, tag="bk")
            if n_masks:
                mask_sb = cpool.tile([KB, n_masks, QB], bf16, tag="mask")
            qb_sb = big.tile([HD, h_loc, Sn], bf16, tag="qb")
            kb_sb = big.tile([HD, h_loc, Sn], bf16, tag="kb")
            v_sb = big.tile([KB, NMB, W], bf16, tag="v")
            wo_sb = big.tile([HD, h_loc, Dm], bf16, tag="wo")
            wq_sb = wqkpool.tile([128, KK, W], bf16, tag="wqr")
            wk_sb = wqkpool.tile([128, KK, W], bf16, tag="wkr")

            xpool_cm = tc.tile_pool(name="xp", bufs=4)
            xpool = xpool_cm.__enter__()
            xTr = xT.rearrange("(kk p) s -> p kk s", p=128)
            xq_tiles = []

            def load_x(sb, chunked=False):
                t = xpool.tile([128, KK, QB], bf16, tag="xt",
                               name=f"x{len(xq_tiles)}")
                if chunked:   # 4 DMAs so the first kk-chunk lands fast
                    for c in range(4):
                        nc.sync.dma_start(t[:, ts(c, 4), :],
                                          xTr[:, ts(c, 4), ts(sb, QB)])
                else:
                    nc.sync.dma_start(t[:], xTr[:, :, ts(sb, QB)])
                xq_tiles.append(t)
                return t

            # Shared projection PSUM pool: one tag, 4 slots of 2 banks.
            # The slot ring carries across phase V and phase QK, so each
            # allocation's previous user was evicted ~2 sub-passes earlier
            # and phase transitions never wait on PSUM banks.
            pj_cm = tc.tile_pool(name="pj", bufs=4, space="PSUM")
            pj = pj_cm.__enter__()

            # ---------------- Phase V: v projection ----------------
            with (
                tc.tile_pool(name="wvp", bufs=1) as wvp,
            ):
                wv_sb = wvp.tile([128, KK, W], bf16, tag="wv")
                wvr = wv.rearrange("(kk p) w -> p kk w", p=128)
                # x0 and wv chunks strictly alternate on ONE queue so the
                # serial DMA_ENGINES resource feeds the kk-loop in order
                t0 = xpool.tile([128, KK, QB], bf16, tag="xt", name="x0")
                for c in range(4):
                    nc.sync.dma_start(t0[:, ts(c, 4), :],
                                      xTr[:, ts(c, 4), ts(0, QB)])
                    nc.sync.dma_start(wv_sb[:, ts(c, 4), :],
                                      wvr[:, ts(c, 4), :])
                xq_tiles.append(t0)
                xts = [t0]
                xts.append(load_x(1))
                nc.scalar.dma_start(bq_sb[:], bq[:])
                nc.scalar.dma_start(bk_sb[:], bk[:])

                for sb in range(NSB):
                    if sb >= 2:
                        xts.append(load_x(sb))
                    xt = xts[sb]
                    for half in range(2):        # m in {0,1} then {2,3}
                        v_ps = pj.tile([KB, 2, W], f32, tag="pj",
                                       name="v_ps")
                        for kk in range(KK):
                            for m2 in range(2):
                                m = half * 2 + m2
                                nc.tensor.matmul(v_ps[:, m2, :],
                                                 xt[:, kk, ts(m, KB)],
                                                 wv_sb[:, kk, :],
                                                 start=(kk == 0),
                                                 stop=(kk == KK - 1))
                        if sb == NSB - 1 and half == 1:
                            xts.append(load_x(0))  # prefetch QK's 1st block
                        for m2 in range(2):
                            m = half * 2 + m2
                            nc.scalar.activation(
                                v_sb[:, sb * (QB // KB) + m, :],
                                v_ps[:, m2, :], AF.Copy)
                    # bulk preloads for later phases: on the SAME queue as
                    # the x stream, behind its configs — queue FIFO order is
                    # the only way to keep their transfers off the x stream's
                    # critical path (DMA_ENGINES is a serial resource)
                    if sb == NSB - 1:
                        nc.sync.dma_start(
                            wq_sb[:],
                            wq.rearrange("(kk p) w -> p kk w", p=128))
                        nc.sync.dma_start(
                            wk_sb[:],
                            wk.rearrange("(kk p) w -> p kk w", p=128))
                        nc.sync.dma_start(cos2_sb[:], cos2[:])
                        nc.sync.dma_start(sinS_sb[:], sinS[:])
                        if n_masks:
                            nc.sync.dma_start(
                                mask_sb[:], pmask.rearrange("n p q -> p n q"))
                        nc.sync.dma_start(
                            wo_sb[:], wo.rearrange("(h d) c -> d h c", d=HD))

            # ---------------- Phase QK: q/k projection + RoPE ------------
            with (
                tc.tile_pool(name="swp", bufs=4) as swp,
            ):
                def rope(srct, h, sb):
                    sl = ts(sb, QB)
                    sw = swp.tile([HD, QB], bf16, tag="sw")
                    nc.gpsimd.dma_start(sw[:64], srct[64:128, h, sl])
                    nc.gpsimd.dma_start(sw[64:128], srct[:64, h, sl])
                    nc.vector.tensor_mul(srct[:, h, sl], srct[:, h, sl],
                                         cos2_sb[:, sl])
                    nc.vector.tensor_mul(sw[:], sw[:], sinS_sb[:, sl])
                    nc.vector.tensor_add(srct[:, h, sl], srct[:, h, sl],
                                         sw[:])

                xts = [xq_tiles[-1]]          # prefetched during phase V
                for sb in range(NSB):
                    if sb >= 1:
                        xts.append(load_x(sb))
                    xt = xts[sb]
                    sl = ts(sb, QB)
                    # four sub-passes (q01, q23, k01, k23): 2 PSUM banks
                    # each, so bias evictions overlap the next sub-pass
                    for wsb, dst, bias in ((wq_sb, qb_sb, bq_sb),
                                           (wk_sb, kb_sb, bk_sb)):
                        for half in range(2):
                            ps = pj.tile([HD, 2, QB], f32, tag="pj",
                                         name="qk_ps")
                            for kk in range(KK):
                                for h2 in range(2):
                                    h = half * 2 + h2
                                    nc.tensor.matmul(ps[:, h2, :],
                                                     wsb[:, kk, ts(h, HD)],
                                                     xt[:, kk, :],
                                                     start=(kk == 0),
                                                     stop=(kk == KK - 1))
                            for h2 in range(2):
                                h = half * 2 + h2
                                nc.scalar.activation(dst[:, h, sl],
                                                     ps[:, h2, :],
                                                     AF.Identity,
                                                     bias=bias[:, h, None])
                            for h2 in range(2):
                                rope(dst, half * 2 + h2, sb)
            xpool_cm.__exit__(None, None, None)
            pj_cm.__exit__(None, None, None)

            # ---------------- Phase B + C: attention + out-proj ----------
            with (
                tc.tile_pool(name="pb", bufs=8) as ppool,
                tc.tile_pool(name="accp", bufs=2) as accp,
                tc.tile_pool(name="lsp", bufs=4) as lsump,
                tc.tile_pool(name="ac", bufs=2) as acache,
                tc.tile_pool(name="oc", bufs=6) as opool,
                tc.tile_pool(name="pss", bufs=2, space="PSUM") as pss,
                tc.tile_pool(name="pso", bufs=2, space="PSUM") as pso,
                tc.tile_pool(name="psc", bufs=2, space="PSUM") as psc,
            ):
                def emit_oproj(qq, act, jobs, drain=False):
                    """One [128q x 512d] out tile: 4 matmuls + copy + store."""
                    if not jobs:
                        return
                    mi_, n = jobs.pop(0)
                    m = qq * (QB // KB) + mi_
                    if drain and (len(jobs) % 2):
                        # final drain: alternate psc with the idle pss banks
                        opw = pss.tile([KB, 2, QB], f32, tag="s", name="opw")
                        op = opw[:, 0, :]
                    else:
                        op = psc.tile([KB, QB], f32, tag="c", name="opc")
                    for h in range(h_loc):
                        nc.tensor.matmul(op[:], act[:, h, ts(mi_, KB)],
                                         wo_sb[:, h, ts(n, QB)],
                                         start=(h == 0),
                                         stop=(h == h_loc - 1))
                    ot = opool.tile([KB, QB], bf16, tag="ot")
                    nc.scalar.activation(ot[:], op[:], AF.Copy)
                    nc.sync.dma_start(out[ts(m, KB), ts(n, QB)], ot[:])

                pending = None     # (qq, act, jobs) from previous q-block
                qorder = sorted(range(NSB), key=lambda q: -len(kb_plan[q]))
                if len(qorder) >= 2:
                    # shortest block first: it warms the pipeline with the
                    # fewest stall-prone groups, and its out-projection then
                    # fills the longest block's many groups one tile per slot
                    qorder = [qorder[-1]] + qorder[:-1]
                for qq in qorder:
                    plan = kb_plan[qq]
                    groups = [plan[i:i + 2] for i in range(0, len(plan), 2)]
                    act = acache.tile([HD, h_loc, QB], bf16, tag="act")
                    qsl = ts(qq, QB)
                    n_slots = max(1, len(groups) * ((h_loc + 1) // 2))
                    per_slot = ((QB // KB) * (Dm // QB) + n_slots - 1) \
                        // n_slots
                    for hpair in ((0, 1), (2, 3))[:max(1, h_loc // 2)]:
                        hpair = [h for h in hpair if h < h_loc]
                        st = {h: (pso.tile([HD, QB], f32, tag="o",
                                           name=f"o{h}"),
                                  accp.tile([KB, QB], f32, tag="acc",
                                            name=f"acc{h}"))
                              for h in hpair}
                        for gi, grp in enumerate(groups):
                            ng = len(grp)
                            # columns < lo are fully masked: skip them in
                            # scores/exp/denominator/PV entirely
                            los = [0 if mi is None else mask_w[mi][0]
                                   for (kbi, mi) in grp]
                            wide = ng == 2 and all(lo == 0 for lo in los)
                            pts = {}
                            for h in hpair:
                                sp2 = pss.tile([KB, 2, QB], f32, tag="s")
                                for s_, (kbi, mi) in enumerate(grp):
                                    lo = los[s_]
                                    nc.tensor.matmul(
                                        sp2[:, s_, lo:],
                                        kb_sb[:, h, ts(kbi, KB)],
                                        qb_sb[:, h,
                                              ds(qq * QB + lo, QB - lo)],
                                        start=True, stop=True)
                                pt = ppool.tile([KB, 2, QB], bf16, tag="p")
                                pts[h] = pt
                                if wide:
                                    nc.scalar.activation(
                                        pt[:, :, :], sp2[:, :, :], AF.Exp,
                                        bias=0.0, scale=scale)
                                else:
                                    for s_ in range(ng):
                                        lo = los[s_]
                                        nc.scalar.activation(
                                            pt[:, s_, lo:], sp2[:, s_, lo:],
                                            AF.Exp, bias=0.0, scale=scale)
                                acc = st[h][1]
                                for s_, (kbi, mi) in enumerate(grp):
                                    if mi is not None:
                                        lo2, hi = mask_w[mi]
                                        if hi > lo2:
                                            nc.vector.tensor_mul(
                                                pt[:, s_, lo2:hi],
                                                pt[:, s_, lo2:hi],
                                                mask_sb[:, mi, lo2:hi])
                                for s_ in range(ng):
                                    lo = los[s_]
                                    if gi == 0 and s_ == 0:
                                        if lo == 0:
                                            nc.vector.tensor_copy(
                                                acc[:], pt[:, 0, :])
                                        else:
                                            nc.gpsimd.memset(acc[:], 0.0)
                                            nc.vector.tensor_add(
                                                acc[:, lo:], acc[:, lo:],
                                                pt[:, 0, lo:])
                                    else:
                                        nc.vector.tensor_add(
                                            acc[:, lo:], acc[:, lo:],
                                            pt[:, s_, lo:])
                            # PE filler: out-proj tiles of the previous
                            # q-block run while this group's exps cook
                            if pending is not None:
                                for _ in range(per_slot):
                                    emit_oproj(pending[0], pending[1],
                                               pending[2])
                            last = (gi == len(groups) - 1)
                            for h in hpair:
                                outp = st[h][0]
                                for s_ in range(ng):
                                    lo = los[s_]
                                    nc.tensor.matmul(
                                        outp[:, lo:],
                                        v_sb[:, grp[s_][0], ts(h, HD)],
                                        pts[h][:, s_, lo:],
                                        start=(gi == 0 and s_ == 0),
                                        stop=(last and s_ == ng - 1))
                        for h in hpair:
                            outp, acc = st[h]
                            lsum = lsump.tile([KB, QB], f32, tag="ls")
                            nc.gpsimd.partition_all_reduce(
                                lsum[:], acc[:], 128, bass_isa.ReduceOp.add)
                            recb = lsump.tile([KB, QB], f32, tag="rc")
                            nc.vector.reciprocal(recb[:], lsum[:])
                            nc.vector.scalar_tensor_tensor(
                                act[:, h, :], outp[:], 1.0, recb[:],
                                op0=ALU.mult, op1=ALU.mult)
                    if pending is not None:
                        while pending[2]:
                            emit_oproj(pending[0], pending[1], pending[2])
                    pending = (qq, act,
                               [(mi_, n) for mi_ in range(QB // KB)
                                for n in range(Dm // QB)])
                while pending[2]:
                    emit_oproj(pending[0], pending[1], pending[2],
                               drain=True)

    nc.compile()
    return nc


# ---------------------------------------------------------------------------
# Host side
# ---------------------------------------------------------------------------

def _bf16(a):
    import ml_dtypes
    return np.ascontiguousarray(np.asarray(a).astype(ml_dtypes.bfloat16))


def _rope_tables(Sn):
    inv = 1.0 / (ROPE_BASE ** (np.arange(0, HD, 2, dtype=np.float32) / HD))
    ang = np.arange(Sn, dtype=np.float32)[:, None] * inv[None, :]
    cosT = np.cos(ang).T.astype(np.float32)          # [64, S]
    sinT = np.sin(ang).T.astype(np.float32)
    cos2 = np.concatenate([cosT, cosT], 0)           # [128, S]
    sinS = np.concatenate([-sinT, sinT], 0)
    return _bf16(cos2), _bf16(sinS)


def _classify_mask(mask, Sn):
    """-> (kb_plan, mask_tiles). kb_plan[qq] = [(kb, mask_idx|None)]."""
    nq, nk = Sn // QB, Sn // KB
    plan = []
    uniq = {}
    tiles = []
    for qq in range(nq):
        row = []
        for kb in range(nk):
            sub = mask[qq * QB:(qq + 1) * QB, kb * KB:(kb + 1) * KB]
            if sub.max() <= -200.0:
                continue                      # exp() == 0 exactly: skip
            if np.all(sub == 0.0):
                row.append((kb, None))
                continue
            t = np.ascontiguousarray(np.exp(sub.astype(np.float64))
                                     .astype(np.float32).T)  # [KB, QB]
            key = t.tobytes()
            if key not in uniq:
                uniq[key] = len(tiles)
                tiles.append(t)
            row.append((kb, uniq[key]))
        plan.append(row)
    return plan, tiles


def _mask_widths(tiles):
    """Per tile (q_lo, w_hi): columns < q_lo are all-zero (the whole score
    column is masked out -> skip computing it); columns >= w_hi are exactly
    1.0 (skip the multiply). The multiply covers [q_lo, w_hi)."""
    rs = []
    for t in tiles:
        nonzero = np.where(np.any(t != 0.0, axis=0))[0]
        q_lo = int(nonzero.min()) if len(nonzero) else QB
        q_lo = (q_lo // 128) * 128
        not_one = np.where(np.any(t != 1.0, axis=0))[0]
        if len(not_one) == 0:
            w_hi = q_lo
        else:
            w_hi = min(QB, ((int(not_one.max()) + 1 + 127) // 128) * 128)
        rs.append((q_lo, w_hi))
    return rs


_CACHE = {}


def _get_runner(plan_key, Sn, Dm, h_loc, kb_plan, mask_w):
    if plan_key in _CACHE:
        return _CACHE[plan_key]
    nc = build_core_program(Sn, Dm, h_loc, kb_plan, mask_w)
    runner = _make_pjrt_runner(nc, N_CORES)
    _CACHE[plan_key] = runner
    return runner


def _make_pjrt_runner(nc, n_cores):
    """Persistent jitted SPMD executor (replicates bass2jax.run_bass_via_pjrt
    multi-core path, but reusable across calls for stable timing)."""
    import jax
    from jax.sharding import Mesh, PartitionSpec
    from jax.experimental.shard_map import shard_map
    from concourse.bass2jax import (_bass_exec_p, install_neuronx_cc_hook,
                                    partition_id_tensor)

    install_neuronx_cc_hook()
    pname = nc.partition_id_tensor.name if nc.partition_id_tensor else None
    in_names, out_names, out_avals, zero_outs = [], [], [], []
    for alloc in nc.m.functions[0].allocations:
        if not isinstance(alloc, mybir.MemoryLocationSet):
            continue
        name = alloc.memorylocations[0].name
        if alloc.kind == "ExternalInput":
            if name != pname:
                in_names.append(name)
        elif alloc.kind == "ExternalOutput":
            shape = tuple(alloc.tensor_shape)
            dtype = mybir.dt.np(alloc.dtype)
            out_names.append(name)
            out_avals.append(jax.core.ShapedArray(shape, dtype))
            zero_outs.append(np.zeros(shape, dtype))
    n_params = len(in_names)
    all_names = in_names + out_names
    if pname is not None:
        all_names = all_names + [pname]

    def _body(*args):
        operands = list(args)
        if pname is not None:
            operands.append(partition_id_tensor())
        outs = _bass_exec_p.bind(
            *operands, out_avals=tuple(out_avals), in_names=tuple(all_names),
            out_names=tuple(out_names), lowering_input_output_aliases=(),
            sim_require_finite=True, sim_require_nnan=True, nc=nc)
        return tuple(outs)

    devices = jax.devices()[:n_cores]
    mesh = Mesh(np.asarray(devices), ("core",))
    nin = n_params + len(out_names)
    jfn = jax.jit(shard_map(_body, mesh=mesh,
                            in_specs=(PartitionSpec("core"),) * nin,
                            out_specs=(PartitionSpec("core"),) * len(out_names),
                            check_rep=False),
                  keep_unused=True)

    def run(in_maps):
        concat = [np.concatenate([np.asarray(m[nm]) for m in in_maps], axis=0)
                  for nm in in_names]
        zeros = [np.zeros((n_cores * z.shape[0], *z.shape[1:]), z.dtype)
                 for z in zero_outs]
        outs = jfn(*concat, *zeros)
        return [{nm: np.asarray(outs[i]).reshape(n_cores, *out_avals[i].shape)[c]
                 for i, nm in enumerate(out_names)} for c in range(n_cores)]

    def make_chain(n):
        def _chain(*args):
            ins = list(args[:n_params])
            outs = tuple(args[n_params:])
            for _ in range(n):
                outs = _body(*ins, *outs)
            return outs
        return jax.jit(shard_map(_chain, mesh=mesh,
                                 in_specs=(PartitionSpec("core"),) * nin,
                                 out_specs=(PartitionSpec("core"),)
                                 * len(out_names),
                                 check_rep=False),
                       keep_unused=True)

    run.jfn = jfn
    run.make_chain = make_chain
    run.in_names = in_names
    run.out_names = out_names
    run.zero_outs = zero_outs
    run.nc = nc
    return run


def _prep_in_maps(x, attn_mask, Wq, bq, Wk, bk, Wv, bv, Wo, mask_tiles):
    cos2, sinS = _rope_tables(S)
    Wg = H_LOC * HD
    pm = _bf16(np.stack(mask_tiles, 0)) if mask_tiles else None
    in_maps = []
    for c in range(N_CORES):
        b, g = divmod(c, N_GROUPS)
        cs = slice(g * Wg, (g + 1) * Wg)
        m = {
            "xT": _bf16(x[b].T),
            "wq": _bf16(Wq[:, cs]),
            "wk": _bf16(Wk[:, cs]),
            "wv": _bf16(Wv[:, cs]),
            "wo": _bf16(Wo[cs, :]),
            "bq": np.ascontiguousarray(
                np.asarray(bq)[cs].reshape(H_LOC, HD).T.astype(np.float32)),
            "bk": np.ascontiguousarray(
                np.asarray(bk)[cs].reshape(H_LOC, HD).T.astype(np.float32)),
            "cos2": cos2,
            "sinS": sinS,
        }
        if pm is not None:
            m["pmask"] = pm
        in_maps.append(m)
    return in_maps


def kernel(x, attn_mask, Wq, bq, Wk, bk, Wv, bv, Wo, bo):
    x = np.asarray(x, dtype=np.float32)
    mask = np.asarray(attn_mask, dtype=np.float32).reshape(S, S)
    kb_plan, mask_tiles = _classify_mask(mask, S)
    mask_w = _mask_widths(mask_tiles)
    plan_key = (tuple(tuple(r) for r in kb_plan), len(mask_tiles))
    runner = _get_runner(plan_key, S, D, H_LOC, kb_plan, mask_w)
    in_maps = _prep_in_maps(x, mask, np.asarray(Wq), np.asarray(bq),
                            np.asarray(Wk), np.asarray(bk), np.asarray(Wv),
                            np.asarray(bv), np.asarray(Wo), mask_tiles)
    results = runner(in_maps)
    # bo' = bo + bv @ Wo (v-bias commutes through softmax normalization)
    bo_eff = (np.asarray(bo, np.float64)
              + np.asarray(bv, np.float64) @ np.asarray(Wo, np.float64)
              ).astype(np.float32)
    out = np.empty((B, S, D), np.float32)
    for b in range(B):
        acc = results[b * N_GROUPS]["out"].astype(np.float32)
        for g in range(1, N_GROUPS):
            acc = acc + results[b * N_GROUPS + g]["out"].astype(np.float32)
        out[b] = acc + bo_eff[None, :]
    return out
